# revision 19
# baseline (speedup 1.0000x reference)
"""CrossNetMix (DCN-v2 MoE cross network) Trainium2 kernel.

Reference math (per layer i, experts e):
    gate = softmax(x_l @ gating_w.T)                       # [B, E]
    v    = tanh(x_l @ V[i,e]); v = tanh(C[i,e] @ v)        # [B, E, R]
    uv   = v @ U[i,e].T                                    # [B, E, D]
    x_l += x0 * (sum_e gate_e * uv_e + bias[i])

Strategy: data-parallel over 8 cores (B/8 rows each); transposed layout
xT [D, B] so the PE contracts over D on partitions.  All matmul operands
in bf16 (PSUM accumulation stays fp32).  Softmax normalization runs in
the DVE 32x32-transposed domain (a [4,BT] op uses 4/128 lanes; transposed
it uses 32 lanes at 1/32 the free size), with exp/tanh sharing one Act
table (no table reloads).  Two batch tiles are software-pipelined per
layer so the PE never idles: while tile A waits on its tanh/gating chain,
the PE runs tile B's contraction matmuls.  State x_l is carried in bf16;
the PSUM drain per d-chunk is an Act copy to bf16 plus cheap 2x-mode DVE
ops, with some multiplies offloaded to the Pool engine.  Inputs/outputs
are host-pre-tiled so every DMA is per-partition contiguous (128 large
descriptors instead of ~1k small ones).
"""

import sys

sys.path.insert(0, "/opt/trn_rl_repo")

import numpy as np
from ml_dtypes import bfloat16

import concourse.bass as bass
import concourse.tile as tile
from concourse import mybir
from concourse.bass_utils import run_bass_kernel_spmd

L, E, D, R, B = 3, 4, 1024, 64, 32768
N_CORES = 8
BC = B // N_CORES          # batch rows per core
BT = 512                   # batch-tile (columns of xT) per PSUM pass
NT = BC // BT              # batch tiles per core
NP = NT // 2               # software-pipelined tile pairs
DC = D // 128              # d-chunks (contraction and output chunks)
TW = DC * BT               # flat elements per tile per partition
F32 = mybir.dt.float32
BF16 = mybir.dt.bfloat16
AF = mybir.ActivationFunctionType
ALU = mybir.AluOpType


def build_nc(with_bias: bool):
    nc = bass.Bass()
    xtb = nc.dram_tensor("xtb", [128, NT * TW], BF16, kind="ExternalInput")
    wv = nc.dram_tensor("wv", [128, L * 2 * DC * 128], BF16, kind="ExternalInput")
    wg = nc.dram_tensor("wg", [128, L * DC * 4], BF16, kind="ExternalInput")
    wc = nc.dram_tensor("wc", [128, L * 2 * 128], BF16, kind="ExternalInput")
    wu = nc.dram_tensor("wu", [128, L * 2 * DC * 128], BF16, kind="ExternalInput")
    wsel = nc.dram_tensor("wsel", [4, 2 * 128], BF16, kind="ExternalInput")
    if with_bias:
        wb1 = nc.dram_tensor("wb1", [128, DC], F32, kind="ExternalInput")    # 1+bias[0]
        wbias = nc.dram_tensor("wbias", [128, L * DC], F32, kind="ExternalInput")
    yt = nc.dram_tensor("yt", [128, NT * TW], BF16, kind="ExternalOutput")

    with tile.TileContext(nc) as tc:
        import contextlib

        ctx = contextlib.ExitStack()
        with ctx:
            singles = ctx.enter_context(tc.tile_pool(name="singles", bufs=1))
            xpool = ctx.enter_context(tc.tile_pool(name="xpool", bufs=6))
            mpool = ctx.enter_context(tc.tile_pool(name="mpool", bufs=6))
            vpool = ctx.enter_context(tc.tile_pool(name="vpool", bufs=3))
            gpool = ctx.enter_context(tc.tile_pool(name="gpool", bufs=3))
            tbpool = ctx.enter_context(tc.tile_pool(name="tbpool", bufs=8))
            mmpool = ctx.enter_context(tc.tile_pool(name="mmpool", bufs=6))
            ps_vc = ctx.enter_context(tc.tile_pool(name="ps_vc", bufs=2, space="PSUM"))
            ps_g = ctx.enter_context(tc.tile_pool(name="ps_g", bufs=1, space="PSUM"))
            ps_w = ctx.enter_context(tc.tile_pool(name="ps_w", bufs=2, space="PSUM"))
            ps_mc = ctx.enter_context(tc.tile_pool(name="ps_mc", bufs=3, space="PSUM"))

            GROUPS = [(0, 1, 2), (3, 4, 5), (6, 7)]
            x0_tiles = {}

            def fetch_tile(t):
                x0 = xpool.tile([128, DC, BT], BF16, tag="x0", name=f"x0_{t}")
                nc.sync.dma_start(
                    out=x0.rearrange("q c b -> q (c b)"),
                    in_=xtb[:, t * TW:(t + 1) * TW],
                )
                return x0

            def prefetch(g):
                if g < len(GROUPS):
                    x0_tiles[g] = [fetch_tile(t) for t in GROUPS[g]]

            # ---- resident weights (flat contiguous DMAs: 128 descriptors),
            # ordered so the first compute tile's inputs land first ----
            gw = singles.tile([128, L, DC, 4], BF16)
            nc.sync.dma_start(out=gw.rearrange("q l c e -> q (l c e)"), in_=wg[:, :])
            cw = singles.tile([128, L, 2, 128], BF16)
            nc.sync.dma_start(out=cw.rearrange("q l p m -> q (l p m)"), in_=wc[:, :])
            sel = singles.tile([4, 2, 128], BF16)
            nc.sync.dma_start(out=sel.rearrange("q p m -> q (p m)"), in_=wsel[:, :])
            if with_bias:
                b1 = singles.tile([128, DC], F32)
                nc.sync.dma_start(out=b1, in_=wb1[:, :])
                bln = singles.tile([128, L * DC], F32)
                nc.sync.dma_start(out=bln, in_=wbias[:, :])
            t0 = fetch_tile(0)
            vw = singles.tile([128, L, 2, DC, 128], BF16)
            nc.sync.dma_start(out=vw.rearrange("q l p c m -> q (l p c m)"), in_=wv[:, :])
            t1 = fetch_tile(1)
            uw = singles.tile([128, L, 2, DC, 128], BF16)
            nc.sync.dma_start(out=uw.rearrange("q l k c m -> q (l k c m)"), in_=wu[:, :])
            t2 = fetch_tile(2)
            x0_tiles[0] = [t0, t1, t2]
            prefetch(1)

            for p in range(len(GROUPS)):
                x0s = x0_tiles.pop(p)
                G = len(x0s)
                cur = list(x0s)
                for l in range(L):
                    last = l == L - 1
                    if last:
                        prefetch(p + 2)
                    v2s = [None] * G
                    wns = [None] * G
                    v2t = [None] * G
                    for s in range(G):
                        cs = cur[s]
                        # gating scores (narrow output, contracts over D)
                        psc = ps_g.tile([4, BT], F32, tag="g", name=f"psc{s}")
                        for c in range(DC):
                            nc.tensor.matmul(
                                psc, gw[:, l, c, :], cs[:, c, :],
                                start=(c == 0), stop=(c == DC - 1),
                            )
                        we = gpool.tile([32, BT], BF16, tag="we", name=f"we{s}")
                        nc.scalar.activation(we[0:4, :], psc, AF.Exp)
                        # softmax normalization in the 32x32-transposed domain
                        # weT[i, k, j] = we[j, 32k+i]; rows j>=4 carry garbage
                        # that is never read back.
                        weT = gpool.tile([32, BT // 32, 32], BF16, tag="weT", name=f"weT{s}")
                        nc.vector.transpose(weT, we)
                        zp = gpool.tile([32, BT // 32, 2], F32, tag="zp", name=f"zp{s}")
                        nc.vector.tensor_add(zp[:, :, 0], weT[:, :, 0], weT[:, :, 1])
                        nc.vector.tensor_add(zp[:, :, 1], weT[:, :, 2], weT[:, :, 3])
                        zT = gpool.tile([32, BT // 32], F32, tag="zT", name=f"zT{s}")
                        nc.vector.tensor_add(zT, zp[:, :, 0], zp[:, :, 1])
                        rzT = gpool.tile([32, BT // 32], F32, tag="rzT", name=f"rzT{s}")
                        nc.vector.reciprocal(rzT, zT)
                        wnT = gpool.tile([32, BT // 32, 32], BF16, tag="wnT", name=f"wnT{s}")
                        for j in range(4):
                            nc.vector.tensor_mul(wnT[:, :, j], weT[:, :, j], rzT)
                        wn = gpool.tile([32, BT], BF16, tag="wn", name=f"wn{s}")
                        nc.vector.transpose(wn, wnT)
                        wns[s] = wn
                        pv0 = ps_vc.tile([128, BT], F32, tag="vc", name=f"pv0{s}")
                        for c in range(DC):
                            nc.tensor.matmul(
                                pv0, vw[:, l, 0, c, :], cs[:, c, :],
                                start=(c == 0), stop=(c == DC - 1),
                            )
                        pv1 = ps_vc.tile([128, BT], F32, tag="vc", name=f"pv1{s}")
                        for c in range(DC):
                            nc.tensor.matmul(
                                pv1, vw[:, l, 1, c, :], cs[:, c, :],
                                start=(c == 0), stop=(c == DC - 1),
                            )
                        v1 = vpool.tile([128, 2, BT], BF16, tag="v1", name=f"v1{s}")
                        nc.scalar.activation(v1[:, 0, :], pv0, AF.Tanh)
                        nc.scalar.activation(v1[:, 1, :], pv1, AF.Tanh)
                        pc0 = ps_vc.tile([128, BT], F32, tag="vc", name=f"pc0{s}")
                        nc.tensor.matmul(pc0, cw[:, l, 0, :], v1[:, 0, :], start=True, stop=True)
                        pc1 = ps_vc.tile([128, BT], F32, tag="vc", name=f"pc1{s}")
                        nc.tensor.matmul(pc1, cw[:, l, 1, :], v1[:, 1, :], start=True, stop=True)
                        v2 = vpool.tile([128, 2, BT], BF16, tag="v2", name=f"v2{s}")
                        nc.scalar.activation(v2[:, 0, :], pc0, AF.Tanh)
                        nc.scalar.activation(v2[:, 1, :], pc1, AF.Tanh)
                        v2t[s] = v2
                    # gate broadcast to the 2x128 expert-row layout (hoisted after
                    # both streams' contraction blocks so the gate chain is off the
                    # PE critical path)
                    for s in range(G):
                        pw0 = ps_w.tile([128, BT], F32, tag="w", name=f"pw0{s}")
                        nc.tensor.matmul(pw0, sel[:, 0, :], wns[s][0:4, :], start=True, stop=True)
                        pw1 = ps_w.tile([128, BT], F32, tag="w", name=f"pw1{s}")
                        nc.tensor.matmul(pw1, sel[:, 1, :], wns[s][0:4, :], start=True, stop=True)
                        vs = vpool.tile([128, 2, BT], BF16, tag="v2s", name=f"v2s{s}")
                        nc.vector.tensor_mul(vs[:, 0, :], v2t[s][:, 0, :], pw0)
                        nc.vector.tensor_mul(vs[:, 1, :], v2t[s][:, 1, :], pw1)
                        v2s[s] = vs
                    # U-stage accumulate + state update
                    for s in range(G):
                        xn = mpool.tile([128, DC, BT], BF16, tag="x", name=f"xn{s}")
                        for c in range(DC):
                            pm = ps_mc.tile([128, BT], F32, tag="mc", name=f"pm{s}")
                            nc.tensor.matmul(pm, uw[:, l, 0, c, :], v2s[s][:, 0, :],
                                             start=True, stop=False)
                            nc.tensor.matmul(pm, uw[:, l, 1, c, :], v2s[s][:, 1, :],
                                             start=False, stop=True)
                            if l == 0:
                                sc = b1[:, c:c + 1] if with_bias else 1.0
                                nc.vector.scalar_tensor_tensor(
                                    xn[:, c, :], pm, sc, x0s[s][:, c, :],
                                    op0=ALU.add, op1=ALU.mult,
                                )
                            elif not with_bias and (c >= 6 or (last and c >= 4)):
                                # direct DVE drain: on each group's last layer
                                # two extra chunks go this way so the Act copy
                                # backlog is short when the next group's Exp
                                # needs the engine (Exp holds the gating PSUM
                                # bank via the pool rotation)
                                m = mmpool.tile([128, BT], BF16, tag="m", name=f"m{s}")
                                nc.vector.tensor_mul(m, pm, x0s[s][:, c, :])
                                nc.vector.tensor_add(xn[:, c, :], cur[s][:, c, :], m)
                            else:
                                tb = tbpool.tile([128, BT], BF16, tag="tb", name=f"tb{s}")
                                if with_bias:
                                    nc.scalar.activation(
                                        tb, pm, AF.Identity,
                                        bias=bln[:, l * DC + c:l * DC + c + 1],
                                    )
                                else:
                                    nc.scalar.copy(tb, pm)
                                m = mmpool.tile([128, BT], BF16, tag="m", name=f"m{s}")
                                nc.vector.tensor_mul(m, tb, x0s[s][:, c, :])
                                nc.vector.tensor_add(xn[:, c, :], cur[s][:, c, :], m)
                            if last:
                                t = GROUPS[p][s]
                                nc.sync.dma_start(
                                    out=yt[:, t * TW + c * BT:t * TW + (c + 1) * BT],
                                    in_=xn[:, c, :],
                                )
                        if not last:
                            cur[s] = xn
    return nc


_split_ctr = [0]


def split_multi_waits(nc):
    """This walrus build accepts only one sync-wait per instruction; hoist
    extra waits onto same-engine NoOps placed just before the instruction."""
    for f in nc.m.functions:
        for bb in f.blocks:
            insts = list(bb.instructions)
            new = []
            changed = False
            for inst in insts:
                si = inst.sync_info
                if si is not None and si.on_wait is not None and len(si.on_wait) > 1:
                    waits = list(si.on_wait)
                    for w in waits[:-1]:
                        _split_ctr[0] += 1
                        nop = mybir.InstNoOp(
                            name=f"I-waitsplit-{_split_ctr[0]}", ins=[], outs=[]
                        )
                        nop.engine = inst.engine
                        nop.sync_info = mybir.SyncInfo(on_wait=[w], on_update=[])
                        new.append(nop)
                    si.on_wait = waits[-1:]
                    changed = True
                new.append(inst)
            if changed:
                bb.instructions = new


def _host_weights(U, V, C, gating_w, bias):
    """Pack params into partition-major SBUF layouts (see build_nc tiles)."""
    # vw[q, l, p, c, m] = V[l, 2p + m//64, c*128+q, m%64]
    Vt = V.reshape(L, 2, 2, D, R)                       # [l, p, eloc, d, r]
    vv = Vt.transpose(3, 0, 1, 2, 4).reshape(D, L, 2, 128)   # [d, l, p, (eloc r)]
    vw = np.ascontiguousarray(
        vv.reshape(DC, 128, L, 2, 128).transpose(1, 2, 3, 0, 4)
    )
    # gw[q, l, c, e] = gating_w[e, c*128+q]
    gwt = gating_w.T.reshape(DC, 128, E)                # [c, q, e]
    gw = np.ascontiguousarray(
        np.broadcast_to(gwt[None], (L, DC, 128, E)).transpose(2, 0, 1, 3)
    )
    # cw[q, l, p, m]: block-diag of C[l,2p].T, C[l,2p+1].T
    cw = np.zeros((128, L, 2, 128), np.float32)
    for l in range(L):
        for p in range(2):
            for el in range(2):
                cw[el * 64:(el + 1) * 64, l, p, el * 64:(el + 1) * 64] = C[l, 2 * p + el].T
    # uw[q, l, k, c, m] = U[l, 2k + q//64, c*128+m, q%64]
    Ut = U.reshape(L, 2, 2, D, R)                       # [l, k, eloc, d, r]
    uu = Ut.transpose(2, 4, 0, 1, 3).reshape(128, L, 2, D)   # [(eloc r), l, k, d]
    uw = np.ascontiguousarray(uu.reshape(128, L, 2, DC, 128))
    # sel[e, p, m] = 1 if 2p + m//64 == e
    sel = np.zeros((4, 2, 128), np.float32)
    for p in range(2):
        for el in range(2):
            sel[2 * p + el, p, el * 64:(el + 1) * 64] = 1.0
    out = {
        "wv": np.ascontiguousarray(vw.reshape(128, -1)).astype(bfloat16),
        "wg": np.ascontiguousarray(gw.reshape(128, -1)).astype(bfloat16),
        "wc": np.ascontiguousarray(cw.reshape(128, -1)).astype(bfloat16),
        "wu": np.ascontiguousarray(uw.reshape(128, -1)).astype(bfloat16),
        "wsel": np.ascontiguousarray(sel.reshape(4, -1)).astype(bfloat16),
    }
    if np.any(bias):
        out["wb1"] = np.ascontiguousarray(
            (1.0 + bias[0]).reshape(DC, 128).T.astype(np.float32)
        )
        out["wbias"] = np.ascontiguousarray(
            bias.reshape(L, DC, 128).transpose(2, 0, 1).reshape(128, L * DC)
        ).astype(np.float32)
    return out


_cache = {}


def kernel(inputs, U, V, C, gating_w, bias):
    inputs = np.asarray(inputs, np.float32)
    U, V, C = np.asarray(U, np.float32), np.asarray(V, np.float32), np.asarray(C, np.float32)
    gating_w, bias = np.asarray(gating_w, np.float32), np.asarray(bias, np.float32)
    with_bias = bool(np.any(bias))

    if with_bias not in _cache:
        nc = build_nc(with_bias)
        split_multi_waits(nc)
        _cache[with_bias] = nc
    nc = _cache[with_bias]

    wmap = _host_weights(U, V, C, gating_w, bias)
    in_maps = []
    for k in range(N_CORES):
        # tile the input so each (partition, tile) DMA row is contiguous:
        # xtb[q, t, c, b] = x.T[c*128+q, t*BT+b]
        xk = inputs[k * BC:(k + 1) * BC].T.reshape(DC, 128, NT, BT)
        xk = np.ascontiguousarray(xk.transpose(1, 2, 0, 3)).astype(bfloat16)
        in_maps.append({"xtb": xk.reshape(128, -1), **wmap})

    res = run_bass_kernel_spmd(
        nc, in_maps, core_ids=list(range(N_CORES)),
        trace=bool(_cache.get("trace")),
    )
    _cache["last_result"] = res
    out = np.empty((B, D), np.float32)
    for k in range(N_CORES):
        yk = res.results[k]["yt"].reshape(128, NT, DC, BT).astype(np.float32)
        out[k * BC:(k + 1) * BC] = yk.transpose(1, 3, 2, 0).reshape(BC, D)
    return out


# revision 20
# speedup vs baseline: 1.0925x; 1.0925x over previous
"""CrossNetMix (DCN-v2 MoE cross network) Trainium2 kernel.

Reference math (per layer i, experts e):
    gate = softmax(x_l @ gating_w.T)                       # [B, E]
    v    = tanh(x_l @ V[i,e]); v = tanh(C[i,e] @ v)        # [B, E, R]
    uv   = v @ U[i,e].T                                    # [B, E, D]
    x_l += x0 * (sum_e gate_e * uv_e + bias[i])

Strategy: data-parallel over 8 cores (B/8 rows each); transposed layout
xT [D, B] so the PE contracts over D on partitions.  All matmul operands
in bf16 (PSUM accumulation stays fp32).  Softmax normalization runs in
the DVE 32x32-transposed domain (a [4,BT] op uses 4/128 lanes; transposed
it uses 32 lanes at 1/32 the free size), with exp/tanh sharing one Act
table (no table reloads).  Two batch tiles are software-pipelined per
layer so the PE never idles: while tile A waits on its tanh/gating chain,
the PE runs tile B's contraction matmuls.  State x_l is carried in bf16;
the PSUM drain per d-chunk is an Act copy to bf16 plus cheap 2x-mode DVE
ops, with some multiplies offloaded to the Pool engine.  Inputs/outputs
are host-pre-tiled so every DMA is per-partition contiguous (128 large
descriptors instead of ~1k small ones).
"""

import sys

sys.path.insert(0, "/opt/trn_rl_repo")

import numpy as np
from ml_dtypes import bfloat16

import concourse.bass as bass
import concourse.tile as tile
from concourse import mybir
from concourse.bass_utils import run_bass_kernel_spmd

L, E, D, R, B = 3, 4, 1024, 64, 32768
N_CORES = 8
BC = B // N_CORES          # batch rows per core
BT = 512                   # batch-tile (columns of xT) per PSUM pass
NT = BC // BT              # batch tiles per core
NP = NT // 2               # software-pipelined tile pairs
DC = D // 128              # d-chunks (contraction and output chunks)
TW = DC * BT               # flat elements per tile per partition
F32 = mybir.dt.float32
BF16 = mybir.dt.bfloat16
AF = mybir.ActivationFunctionType
ALU = mybir.AluOpType


def build_nc(with_bias: bool):
    nc = bass.Bass()
    xtb = nc.dram_tensor("xtb", [128, NT * TW], BF16, kind="ExternalInput")
    wv = nc.dram_tensor("wv", [128, L * 2 * DC * 128], BF16, kind="ExternalInput")
    wg = nc.dram_tensor("wg", [128, L * DC * 4], BF16, kind="ExternalInput")
    wc = nc.dram_tensor("wc", [128, L * 2 * 128], BF16, kind="ExternalInput")
    wu = nc.dram_tensor("wu", [128, L * 2 * DC * 128], BF16, kind="ExternalInput")
    wsel = nc.dram_tensor("wsel", [4, 2 * 128], BF16, kind="ExternalInput")
    if with_bias:
        wb1 = nc.dram_tensor("wb1", [128, DC], F32, kind="ExternalInput")    # 1+bias[0]
        wbias = nc.dram_tensor("wbias", [128, L * DC], F32, kind="ExternalInput")
    yt = nc.dram_tensor("yt", [128, NT * TW], BF16, kind="ExternalOutput")

    with tile.TileContext(nc) as tc:
        import contextlib

        ctx = contextlib.ExitStack()
        with ctx:
            singles = ctx.enter_context(tc.tile_pool(name="singles", bufs=1))
            xpool = ctx.enter_context(tc.tile_pool(name="xpool", bufs=6))
            mpool = ctx.enter_context(tc.tile_pool(name="mpool", bufs=6))
            vpool = ctx.enter_context(tc.tile_pool(name="vpool", bufs=3))
            gpool = ctx.enter_context(tc.tile_pool(name="gpool", bufs=3))
            tbpool = ctx.enter_context(tc.tile_pool(name="tbpool", bufs=8))
            mmpool = ctx.enter_context(tc.tile_pool(name="mmpool", bufs=6))
            ps_vc = ctx.enter_context(tc.tile_pool(name="ps_vc", bufs=2, space="PSUM"))
            ps_g = ctx.enter_context(tc.tile_pool(name="ps_g", bufs=1, space="PSUM"))
            ps_w = ctx.enter_context(tc.tile_pool(name="ps_w", bufs=2, space="PSUM"))
            ps_mc = ctx.enter_context(tc.tile_pool(name="ps_mc", bufs=3, space="PSUM"))

            GROUPS = [(0, 1, 2), (3, 4, 5), (6, 7)]
            x0_tiles = {}

            def fetch_tile(t):
                x0 = xpool.tile([128, DC, BT], BF16, tag="x0", name=f"x0_{t}")
                nc.sync.dma_start(
                    out=x0.rearrange("q c b -> q (c b)"),
                    in_=xtb[:, t * TW:(t + 1) * TW],
                )
                return x0

            def prefetch(g):
                if g < len(GROUPS):
                    x0_tiles[g] = [fetch_tile(t) for t in GROUPS[g]]

            # ---- resident weights (flat contiguous DMAs: 128 descriptors),
            # ordered so the first compute tile's inputs land first ----
            gw = singles.tile([128, L, DC, 4], BF16)
            nc.sync.dma_start(out=gw.rearrange("q l c e -> q (l c e)"), in_=wg[:, :])
            cw = singles.tile([128, L, 2, 128], BF16)
            nc.sync.dma_start(out=cw.rearrange("q l p m -> q (l p m)"), in_=wc[:, :])
            sel = singles.tile([4, 2, 128], BF16)
            nc.sync.dma_start(out=sel.rearrange("q p m -> q (p m)"), in_=wsel[:, :])
            if with_bias:
                b1 = singles.tile([128, DC], F32)
                nc.sync.dma_start(out=b1, in_=wb1[:, :])
                bln = singles.tile([128, L * DC], F32)
                nc.sync.dma_start(out=bln, in_=wbias[:, :])
            t0 = fetch_tile(0)
            vw = singles.tile([128, L, 2, DC, 128], BF16)
            nc.sync.dma_start(out=vw.rearrange("q l p c m -> q (l p c m)"), in_=wv[:, :])
            t1 = fetch_tile(1)
            uw = singles.tile([128, L, 2, DC, 128], BF16)
            nc.sync.dma_start(out=uw.rearrange("q l k c m -> q (l k c m)"), in_=wu[:, :])
            t2 = fetch_tile(2)
            x0_tiles[0] = [t0, t1, t2]
            prefetch(1)

            for p in range(len(GROUPS)):
                x0s = x0_tiles.pop(p)
                G = len(x0s)
                cur = list(x0s)
                for l in range(L):
                    last = l == L - 1
                    if last:
                        prefetch(p + 2)
                    v2s = [None] * G
                    wns = [None] * G
                    v2t = [None] * G
                    for s in range(G):
                        cs = cur[s]
                        # gating scores (narrow output, contracts over D)
                        psc = ps_g.tile([4, BT], F32, tag="g", name=f"psc{s}")
                        for c in range(DC):
                            nc.tensor.matmul(
                                psc, gw[:, l, c, :], cs[:, c, :],
                                start=(c == 0), stop=(c == DC - 1),
                            )
                        we = gpool.tile([32, BT], BF16, tag="we", name=f"we{s}")
                        nc.scalar.activation(we[0:4, :], psc, AF.Exp)
                        # softmax normalization in the 32x32-transposed domain
                        # weT[i, k, j] = we[j, 32k+i]; rows j>=4 carry garbage
                        # that is never read back.
                        weT = gpool.tile([32, BT // 32, 32], BF16, tag="weT", name=f"weT{s}")
                        nc.vector.transpose(weT, we)
                        zp = gpool.tile([32, BT // 32, 2], F32, tag="zp", name=f"zp{s}")
                        nc.vector.tensor_add(zp[:, :, 0], weT[:, :, 0], weT[:, :, 1])
                        nc.vector.tensor_add(zp[:, :, 1], weT[:, :, 2], weT[:, :, 3])
                        zT = gpool.tile([32, BT // 32], F32, tag="zT", name=f"zT{s}")
                        nc.vector.tensor_add(zT, zp[:, :, 0], zp[:, :, 1])
                        rzT = gpool.tile([32, BT // 32], F32, tag="rzT", name=f"rzT{s}")
                        nc.vector.reciprocal(rzT, zT)
                        wnT = gpool.tile([32, BT // 32, 32], BF16, tag="wnT", name=f"wnT{s}")
                        for j in range(4):
                            nc.vector.tensor_mul(wnT[:, :, j], weT[:, :, j], rzT)
                        wn = gpool.tile([32, BT], BF16, tag="wn", name=f"wn{s}")
                        nc.vector.transpose(wn, wnT)
                        wns[s] = wn
                        pv0 = ps_vc.tile([128, BT], F32, tag="vc", name=f"pv0{s}")
                        for c in range(DC):
                            nc.tensor.matmul(
                                pv0, vw[:, l, 0, c, :], cs[:, c, :],
                                start=(c == 0), stop=(c == DC - 1),
                            )
                        pv1 = ps_vc.tile([128, BT], F32, tag="vc", name=f"pv1{s}")
                        for c in range(DC):
                            nc.tensor.matmul(
                                pv1, vw[:, l, 1, c, :], cs[:, c, :],
                                start=(c == 0), stop=(c == DC - 1),
                            )
                        v1 = vpool.tile([128, 2, BT], BF16, tag="v1", name=f"v1{s}")
                        nc.scalar.activation(v1[:, 0, :], pv0, AF.Tanh)
                        nc.scalar.activation(v1[:, 1, :], pv1, AF.Tanh)
                        pc0 = ps_vc.tile([128, BT], F32, tag="vc", name=f"pc0{s}")
                        nc.tensor.matmul(pc0, cw[:, l, 0, :], v1[:, 0, :], start=True, stop=True)
                        pc1 = ps_vc.tile([128, BT], F32, tag="vc", name=f"pc1{s}")
                        nc.tensor.matmul(pc1, cw[:, l, 1, :], v1[:, 1, :], start=True, stop=True)
                        v2 = vpool.tile([128, 2, BT], BF16, tag="v2", name=f"v2{s}")
                        nc.scalar.activation(v2[:, 0, :], pc0, AF.Tanh)
                        nc.scalar.activation(v2[:, 1, :], pc1, AF.Tanh)
                        v2t[s] = v2
                    # gate broadcast to the 2x128 expert-row layout (hoisted after
                    # both streams' contraction blocks so the gate chain is off the
                    # PE critical path)
                    for s in range(G):
                        pw0 = ps_w.tile([128, BT], F32, tag="w", name=f"pw0{s}")
                        nc.tensor.matmul(pw0, sel[:, 0, :], wns[s][0:4, :], start=True, stop=True)
                        pw1 = ps_w.tile([128, BT], F32, tag="w", name=f"pw1{s}")
                        nc.tensor.matmul(pw1, sel[:, 1, :], wns[s][0:4, :], start=True, stop=True)
                        vs = vpool.tile([128, 2, BT], BF16, tag="v2s", name=f"v2s{s}")
                        nc.vector.tensor_mul(vs[:, 0, :], v2t[s][:, 0, :], pw0)
                        nc.vector.tensor_mul(vs[:, 1, :], v2t[s][:, 1, :], pw1)
                        v2s[s] = vs
                    # U-stage accumulate + state update
                    for s in range(G):
                        xn = mpool.tile([128, DC, BT], BF16, tag="x", name=f"xn{s}")
                        for c in range(DC):
                            pm = ps_mc.tile([128, BT], F32, tag="mc", name=f"pm{s}")
                            nc.tensor.matmul(pm, uw[:, l, 0, c, :], v2s[s][:, 0, :],
                                             start=True, stop=False)
                            nc.tensor.matmul(pm, uw[:, l, 1, c, :], v2s[s][:, 1, :],
                                             start=False, stop=True)
                            if l == 0:
                                sc = b1[:, c:c + 1] if with_bias else 1.0
                                nc.vector.scalar_tensor_tensor(
                                    xn[:, c, :], pm, sc, x0s[s][:, c, :],
                                    op0=ALU.add, op1=ALU.mult,
                                )
                            else:
                                tb = tbpool.tile([128, BT], BF16, tag="tb", name=f"tb{s}")
                                if with_bias:
                                    nc.scalar.activation(
                                        tb, pm, AF.Identity,
                                        bias=bln[:, l * DC + c:l * DC + c + 1],
                                    )
                                else:
                                    nc.scalar.copy(tb, pm)
                                m = mmpool.tile([128, BT], BF16, tag="m", name=f"m{s}")
                                # all-SBUF bf16 mul is legal on Pool; offload half
                                meng = nc.gpsimd if c < 4 else nc.vector
                                meng.tensor_mul(m, tb, x0s[s][:, c, :])
                                nc.vector.tensor_add(xn[:, c, :], cur[s][:, c, :], m)
                            if last:
                                t = GROUPS[p][s]
                                nc.sync.dma_start(
                                    out=yt[:, t * TW + c * BT:t * TW + (c + 1) * BT],
                                    in_=xn[:, c, :],
                                )
                        if not last:
                            cur[s] = xn
    return nc


_split_ctr = [0]


def split_multi_waits(nc):
    """This walrus build accepts only one sync-wait per instruction; hoist
    extra waits onto same-engine NoOps placed just before the instruction."""
    for f in nc.m.functions:
        for bb in f.blocks:
            insts = list(bb.instructions)
            new = []
            changed = False
            for inst in insts:
                si = inst.sync_info
                if si is not None and si.on_wait is not None and len(si.on_wait) > 1:
                    waits = list(si.on_wait)
                    for w in waits[:-1]:
                        _split_ctr[0] += 1
                        nop = mybir.InstNoOp(
                            name=f"I-waitsplit-{_split_ctr[0]}", ins=[], outs=[]
                        )
                        nop.engine = inst.engine
                        nop.sync_info = mybir.SyncInfo(on_wait=[w], on_update=[])
                        new.append(nop)
                    si.on_wait = waits[-1:]
                    changed = True
                new.append(inst)
            if changed:
                bb.instructions = new


def _host_weights(U, V, C, gating_w, bias):
    """Pack params into partition-major SBUF layouts (see build_nc tiles)."""
    # vw[q, l, p, c, m] = V[l, 2p + m//64, c*128+q, m%64]
    Vt = V.reshape(L, 2, 2, D, R)                       # [l, p, eloc, d, r]
    vv = Vt.transpose(3, 0, 1, 2, 4).reshape(D, L, 2, 128)   # [d, l, p, (eloc r)]
    vw = np.ascontiguousarray(
        vv.reshape(DC, 128, L, 2, 128).transpose(1, 2, 3, 0, 4)
    )
    # gw[q, l, c, e] = gating_w[e, c*128+q]
    gwt = gating_w.T.reshape(DC, 128, E)                # [c, q, e]
    gw = np.ascontiguousarray(
        np.broadcast_to(gwt[None], (L, DC, 128, E)).transpose(2, 0, 1, 3)
    )
    # cw[q, l, p, m]: block-diag of C[l,2p].T, C[l,2p+1].T
    cw = np.zeros((128, L, 2, 128), np.float32)
    for l in range(L):
        for p in range(2):
            for el in range(2):
                cw[el * 64:(el + 1) * 64, l, p, el * 64:(el + 1) * 64] = C[l, 2 * p + el].T
    # uw[q, l, k, c, m] = U[l, 2k + q//64, c*128+m, q%64]
    Ut = U.reshape(L, 2, 2, D, R)                       # [l, k, eloc, d, r]
    uu = Ut.transpose(2, 4, 0, 1, 3).reshape(128, L, 2, D)   # [(eloc r), l, k, d]
    uw = np.ascontiguousarray(uu.reshape(128, L, 2, DC, 128))
    # sel[e, p, m] = 1 if 2p + m//64 == e
    sel = np.zeros((4, 2, 128), np.float32)
    for p in range(2):
        for el in range(2):
            sel[2 * p + el, p, el * 64:(el + 1) * 64] = 1.0
    out = {
        "wv": np.ascontiguousarray(vw.reshape(128, -1)).astype(bfloat16),
        "wg": np.ascontiguousarray(gw.reshape(128, -1)).astype(bfloat16),
        "wc": np.ascontiguousarray(cw.reshape(128, -1)).astype(bfloat16),
        "wu": np.ascontiguousarray(uw.reshape(128, -1)).astype(bfloat16),
        "wsel": np.ascontiguousarray(sel.reshape(4, -1)).astype(bfloat16),
    }
    if np.any(bias):
        out["wb1"] = np.ascontiguousarray(
            (1.0 + bias[0]).reshape(DC, 128).T.astype(np.float32)
        )
        out["wbias"] = np.ascontiguousarray(
            bias.reshape(L, DC, 128).transpose(2, 0, 1).reshape(128, L * DC)
        ).astype(np.float32)
    return out


_cache = {}


def kernel(inputs, U, V, C, gating_w, bias):
    inputs = np.asarray(inputs, np.float32)
    U, V, C = np.asarray(U, np.float32), np.asarray(V, np.float32), np.asarray(C, np.float32)
    gating_w, bias = np.asarray(gating_w, np.float32), np.asarray(bias, np.float32)
    with_bias = bool(np.any(bias))

    if with_bias not in _cache:
        nc = build_nc(with_bias)
        split_multi_waits(nc)
        _cache[with_bias] = nc
    nc = _cache[with_bias]

    wmap = _host_weights(U, V, C, gating_w, bias)
    in_maps = []
    for k in range(N_CORES):
        # tile the input so each (partition, tile) DMA row is contiguous:
        # xtb[q, t, c, b] = x.T[c*128+q, t*BT+b]
        xk = inputs[k * BC:(k + 1) * BC].T.reshape(DC, 128, NT, BT)
        xk = np.ascontiguousarray(xk.transpose(1, 2, 0, 3)).astype(bfloat16)
        in_maps.append({"xtb": xk.reshape(128, -1), **wmap})

    res = run_bass_kernel_spmd(
        nc, in_maps, core_ids=list(range(N_CORES)),
        trace=bool(_cache.get("trace")),
    )
    _cache["last_result"] = res
    out = np.empty((B, D), np.float32)
    for k in range(N_CORES):
        yk = res.results[k]["yt"].reshape(128, NT, DC, BT).astype(np.float32)
        out[k * BC:(k + 1) * BC] = yk.transpose(1, 3, 2, 0).reshape(BC, D)
    return out
